# revision 1
# baseline (speedup 1.0000x reference)
"""Trainium2 Bass kernel for nn_MultiHeadGlobalAttention.

Math (B=64, N=4096, C=128, H=4):
  mask[b,n] = n < graph_size[b]
  Vg = (V @ weight + bias).reshape(B,N,H,C)
  a[b,n,h] = sum_c Vg[b,n,h,c] * tune[0,h,c]   -> leaky_relu -> masked softmax over n
  out[b] = (sum_n a[b,n,h] * Vg[b,n,h,:]).reshape(H*C)

Key reduction: softmax weights sum to 1, so
  out[b, h*C:(h+1)*C] = (sum_n e[n,h] * V[b,n,:]) / Z[b,h] @ W[:, h*C:(h+1)*C] + bias[h*C:(h+1)*C]
with logits l[n,h] = V[b,n,:] @ w2[:,h] + b2[h], w2 = sum_d W[:,h*C+d]*tune[h,d],
b2 = sum_d bias[h*C+d]*tune[h,d], e = max(exp(l), exp(ALPHA*l)) * mask01.
Vg is never materialized; V is read once per node.

Sharding: 64 graphs -> 8 cores x 8 slots. Graphs sorted by chunk count
(ceil(gs/128)) descending; rank group g -> slot g, one graph per core.
Slot cap = max chunks in group -> uniform SPMD program across cores.

Device schedule (per-instruction overheads dominate -> batch everything):
  - V shipped partition-major [128, nchunk*128]; loaded in GD-chunk DMAs.
  - logits accumulated into a [128, GE*4] PSUM region per GE-chunk group;
    2 batched Exps + 1 max + 1 mask-mul per group.
  - V^T via PE transpose, 4 chunks per PSUM bank, single copy-back
    alternating DVE/ACT.
  - e-weighted V accumulation matmuls emitted one group late so the
    in-order PE queue never stalls on the exp chain.
"""

import math
import numpy as np

import concourse.bass as bass
import concourse.tile as tile
from concourse import bacc, mybir
from concourse.bass_utils import run_bass_kernel_spmd
from concourse.masks import make_identity

B, N, C, H = 64, 4096, 128, 4
P = 128          # chunk size == partition count
ALPHA = 0.2      # leaky_relu negative slope
NCORES = 8
NSLOTS = B // NCORES
GD = 16          # chunks per V DMA
GE = 32          # chunks per exp/softmax group
F32 = mybir.dt.float32


def _plan(graph_size):
    """Sort graphs by chunk count ascending; rank group g -> slot g across
    cores. Ascending so the big slot is last in the stream: every other
    slot's tail work (Z, sall copy, out matmul, DMA) completes while V is
    still streaming, leaving only the big slot's short chain at the end."""
    nch = np.maximum(1, np.ceil(np.asarray(graph_size, np.int64) / P).astype(np.int64))
    order = np.argsort(nch, kind="stable")
    caps = [int(nch[order[g * NCORES + NCORES - 1]]) for g in range(NSLOTS)]
    offs = np.concatenate([[0], np.cumsum(caps)]).astype(np.int64)
    return order, caps, offs, int(offs[-1])


def _build_program(caps, nchunk):
    nc = bacc.Bacc("TRN2", target_bir_lowering=False, debug=False)

    vh_d = nc.declare_dram_parameter("Vh", [P, nchunk * P], F32, isOutput=False)
    em_d = nc.declare_dram_parameter("em", [P, nchunk * H], F32, isOutput=False)
    wb_d = nc.declare_dram_parameter("wb", [C, 2 * H], F32, isOutput=False)
    wm_d = nc.declare_dram_parameter("wmat", [C, H * C], F32, isOutput=False)
    raw_d = nc.declare_dram_parameter("raw", [NSLOTS * H, H * C], F32, isOutput=True)
    zc_d = nc.declare_dram_parameter("zc", [P, NSLOTS], F32, isOutput=True)

    offs = [0]
    for cp in caps:
        offs.append(offs[-1] + cp)
    slot_of = np.zeros(nchunk, np.int64)
    for g in range(NSLOTS):
        slot_of[offs[g]:offs[g + 1]] = g
    # V DMA group sizes: small ramp-up so the first transposes start early,
    # big middle groups to amortize per-DMA overhead, halving ramp-down so
    # the final chunks arrive incrementally and the tail chain is short.
    sizes = []
    rem = nchunk
    for s in (4, 4, 8):
        if rem <= 0:
            break
        t = min(s, rem)
        sizes.append(t)
        rem -= t
    while rem > 48:
        sizes.append(32)
        rem -= 32
    while rem > 2:
        t = max(2, rem // 2)
        sizes.append(t)
        rem -= t
    if rem:
        sizes.append(rem)
    gstart = [0]
    for s in sizes:
        gstart.append(gstart[-1] + s)
    loc = []
    for k, s in enumerate(sizes):
        for r in range(s):
            loc.append((k, r))
    ngd = len(sizes)
    # softmax group boundaries: 32-wide while plenty remains, halving tail
    geb = [0]
    rem = nchunk
    while rem > 64:
        geb.append(geb[-1] + 32)
        rem -= 32
    while rem > 4:
        t = max(4, rem // 2)
        geb.append(geb[-1] + t)
        rem -= t
    if rem:
        geb.append(geb[-1] + rem)
    nge = len(geb) - 1

    with tile.TileContext(nc) as tc:
        with (
            tc.tile_pool(name="consts", bufs=1) as consts,
            tc.tile_pool(name="vres", bufs=ngd) as vres,
            tc.tile_pool(name="vt4", bufs=3) as vt4p,
            tc.tile_pool(name="e12", bufs=6) as e12p,
            tc.tile_pool(name="eallp", bufs=1) as eall_pool,
            tc.tile_pool(name="outp", bufs=1) as out_pool,
            tc.tile_pool(name="ps_vt", bufs=2, space="PSUM") as ps_vt,
            tc.tile_pool(name="ps_l", bufs=2, space="PSUM") as ps_l,
            tc.tile_pool(name="ps_s", bufs=1, space="PSUM") as ps_s,
            tc.tile_pool(name="ps_o", bufs=2, space="PSUM") as ps_o,
            tc.tile_pool(name="ps_z", bufs=1, space="PSUM") as ps_z,
        ):
            # DMA order matters: transfers share one exclusive device, so
            # order = criticality. wb (w2|b2) gates the first logits; the
            # first (small) V groups gate the first transposes; em gates the
            # first softmax; wm is only needed once slots start completing.
            wb_sb = consts.tile([C, 2 * H], F32)
            nc.sync.dma_start(wb_sb[:], wb_d[:])
            w2_ap = wb_sb[:, 0:H]
            b2_ap = wb_sb[0:1, H:2 * H]

            vg = []
            for k in range(ngd):
                t = vres.tile([P, sizes[k] * P], F32)
                vg.append(t)
            nc.sync.dma_start(vg[0][:], vh_d[:, 0:sizes[0] * P])
            nc.sync.dma_start(
                vg[1][:], vh_d[:, gstart[1] * P:gstart[1] * P + sizes[1] * P]
            )

            em_sb = consts.tile([P, nchunk * H], F32)
            nc.sync.dma_start(em_sb[:], em_d[:])

            for k in range(2, ngd):
                a = gstart[k] * P
                nc.sync.dma_start(vg[k][:], vh_d[:, a:a + sizes[k] * P])

            wm_sb = consts.tile([C, H * C], F32)
            nc.sync.dma_start(wm_sb[:], wm_d[:])

            onesrow = consts.tile([1, P], F32)
            nc.gpsimd.memset(onesrow[:], 1.0)
            ident = consts.tile([P, P], F32)
            make_identity(nc, ident[:])
            ones = consts.tile([P, 1], F32)
            nc.gpsimd.memset(ones[:], 1.0)

            eall_sb = eall_pool.tile([P, nchunk * H], F32)
            psum_s = ps_s.tile([C, NSLOTS * H], F32)
            zc_ps = ps_z.tile([P, NSLOTS], F32)
            zc_sb = out_pool.tile([P, NSLOTS], F32)
            nc.gpsimd.memset(zc_sb[:], 0.0)
            sall = out_pool.tile([C, NSLOTS * H], F32)

            def vsl(j):
                k, r = loc[j]
                return vg[k][:, r * P:(r + 1) * P]

            rtog = [0]

            def emit_ready(c0, c1):
                # accumulation matmuls for chunks [c0, c1), then the full
                # output pipeline (Z partials, sall copy, out matmul, copy,
                # DMA) for any slot whose chunk span completes in this range
                for j in range(c0, c1):
                    g = int(slot_of[j])
                    nc.tensor.matmul(
                        psum_s[:, g * H:(g + 1) * H], vsl(j),
                        eall_sb[:, j * H:(j + 1) * H],
                        start=(j == offs[g]), stop=(j == offs[g + 1] - 1),
                    )
                for g in range(NSLOTS):
                    if not (c0 < offs[g + 1] <= c1):
                        continue
                    lo = offs[g] * H
                    hi = lo + caps[g] * H
                    nc.tensor.matmul(
                        zc_ps[0:caps[g] * H, g:g + 1], eall_sb[:, lo:hi],
                        ones[:], start=True, stop=True,
                    )
                    nc.vector.tensor_copy(
                        zc_sb[0:caps[g] * H, g:g + 1],
                        zc_ps[0:caps[g] * H, g:g + 1],
                    )
                    gl, gh = g * H, (g + 1) * H
                    nc.vector.tensor_copy(sall[:, gl:gh], psum_s[:, gl:gh])
                    if g < NSLOTS - 1:
                        rp = ps_o.tile([H, H * C], F32)
                        rb = out_pool.tile([H, H * C], F32)
                        nc.tensor.matmul(
                            rp[:], sall[:, gl:gh], wm_sb[:],
                            start=True, stop=True,
                        )
                        if rtog[0]:
                            nc.vector.tensor_copy(rb[:], rp[:])
                        else:
                            nc.scalar.activation(
                                rb[:], rp[:], mybir.ActivationFunctionType.Copy
                            )
                        rtog[0] ^= 1
                        nc.sync.dma_start(raw_d[gl:gh, :], rb[:])
                    else:
                        # last slot: per-head matmuls into one PSUM row so
                        # copies/DMA pipeline behind PE; host reads the
                        # diagonal blocks from raw row gl (see _assemble)
                        rp = ps_o.tile([1, H * C], F32)
                        rb = out_pool.tile([1, H * C], F32)
                        for h in range(H):
                            blk = slice(h * C, (h + 1) * C)
                            nc.tensor.matmul(
                                rp[0:1, blk], sall[:, gl + h:gl + h + 1],
                                wm_sb[:, blk], start=True, stop=True,
                            )
                            if h == 1:
                                nc.scalar.activation(
                                    rb[0:1, 0:2 * C], rp[0:1, 0:2 * C],
                                    mybir.ActivationFunctionType.Copy,
                                )
                            elif h == 3:
                                nc.vector.tensor_copy(
                                    rb[0:1, 2 * C:4 * C], rp[0:1, 2 * C:4 * C]
                                )
                        nc.sync.dma_start(zc_d[:], zc_sb[:])
                        nc.sync.dma_start(raw_d[gl:gl + 1, :], rb[:])

            toggle = 0
            pending = None
            for ke in range(nge):
                c0, c1 = geb[ke], geb[ke + 1]
                w = (c1 - c0) * H
                l_ps = ps_l.tile([P, GE * H], F32)
                for q0 in range(c0, c1, 4):
                    q1 = min(c1, q0 + 4)
                    qw = (q1 - q0) * P
                    vt_ps = ps_vt.tile([P, 4 * P], F32)
                    for j in range(q0, q1):
                        nc.tensor.transpose(
                            vt_ps[:, (j - q0) * P:(j - q0 + 1) * P], vsl(j), ident[:]
                        )
                    vt_sb = vt4p.tile([P, 4 * P], F32)
                    if toggle:
                        nc.vector.tensor_copy(vt_sb[:, :qw], vt_ps[:, :qw])
                    else:
                        nc.scalar.activation(
                            vt_sb[:, :qw], vt_ps[:, :qw],
                            mybir.ActivationFunctionType.Copy,
                        )
                    toggle ^= 1
                    for j in range(q0, q1):
                        off = (j - c0) * H
                        nc.tensor.matmul(
                            l_ps[:, off:off + H],
                            vt_sb[:, (j - q0) * P:(j - q0 + 1) * P], w2_ap,
                            start=True, stop=False,
                        )
                        nc.tensor.matmul(
                            l_ps[:, off:off + H], onesrow[:], b2_ap,
                            start=False, stop=True,
                        )
                e1 = e12p.tile([P, GE * H], F32)
                e2 = e12p.tile([P, GE * H], F32)
                m12 = e12p.tile([P, GE * H], F32)
                nc.scalar.activation(
                    e1[:, :w], l_ps[:, :w], mybir.ActivationFunctionType.Exp
                )
                nc.scalar.activation(
                    e2[:, :w], l_ps[:, :w], mybir.ActivationFunctionType.Exp,
                    scale=ALPHA,
                )
                nc.vector.tensor_max(m12[:, :w], e1[:, :w], e2[:, :w])
                nc.vector.tensor_mul(
                    eall_sb[:, c0 * H:c0 * H + w], m12[:, :w],
                    em_sb[:, c0 * H:c0 * H + w],
                )
                # one-group-late accum emission keeps the in-order PE queue
                # from stalling on the exp chain mid-stream; in the last few
                # (small) groups PE is idle anyway, so emit immediately to
                # avoid queueing tail work behind later transposes
                if ke >= nge - 3:
                    if pending is not None:
                        emit_ready(*pending)
                        pending = None
                    emit_ready(c0, c1)
                else:
                    if pending is not None:
                        emit_ready(*pending)
                    pending = (c0, c1)
            if pending is not None:
                emit_ready(*pending)

    nc.compile()
    return nc


def _host_inputs(V, graph_size, weight, bias, tune_weight, order, caps, offs, nchunk):
    tw = np.asarray(tune_weight, np.float32)[0]                      # [H, C]
    wr = np.asarray(weight, np.float32).reshape(C, H, C)
    w2 = np.einsum("chd,hd->ch", wr, tw).astype(np.float32)          # [C, H]
    b2 = np.einsum("hd,hd->h", np.asarray(bias, np.float32).reshape(H, C), tw)
    b2r = b2.astype(np.float32).reshape(1, H)

    wb = np.zeros((C, 2 * H), np.float32)
    wb[:, :H] = w2
    wb[0, H:2 * H] = b2r[0]

    gs = np.asarray(graph_size, np.int64)
    in_maps = []
    core_graphs = []
    for c in range(NCORES):
        graphs = [int(order[g * NCORES + c]) for g in range(NSLOTS)]
        core_graphs.append(graphs)
        vcat = np.concatenate(
            [V[b, : caps[g] * P, :] for g, b in enumerate(graphs)], axis=0
        ).astype(np.float32, copy=False)
        vh = np.ascontiguousarray(
            vcat.reshape(nchunk, P, C).transpose(1, 0, 2).reshape(P, nchunk * C)
        )
        mask = np.zeros((P, nchunk), np.float32)
        prow = np.arange(P)
        for g, b in enumerate(graphs):
            for j in range(caps[g]):
                mask[(j * P + prow) < gs[b], offs[g] + j] = 1.0
        em = np.repeat(mask, H, axis=1)
        in_maps.append(
            {
                "Vh": vh,
                "em": em,
                "wb": wb,
                "wmat": np.ascontiguousarray(np.asarray(weight, np.float32)),
            }
        )
    return in_maps, core_graphs


def _assemble(results, core_graphs, caps, offs, nchunk, bias):
    bias = np.asarray(bias, np.float32)
    out = np.empty((B, H * C), np.float32)
    for c in range(NCORES):
        raw = np.asarray(results[c]["raw"])                    # [NSLOTS*H, H*C]
        zc = np.asarray(results[c]["zc"])                      # [P, NSLOTS]
        for g, b in enumerate(core_graphs[c]):
            Z = zc[: caps[g] * H, g].reshape(caps[g], H).sum(axis=0)   # [H]
            for h in range(H):
                blk = slice(h * C, (h + 1) * C)
                # last slot packs all head blocks into its first raw row
                r = g * H if g == NSLOTS - 1 else g * H + h
                out[b, blk] = raw[r, blk] / Z[h] + bias[blk]
    return out


def kernel(V, graph_size, weight, bias, tune_weight, _run=None):
    order, caps, offs, nchunk = _plan(graph_size)
    nc = _build_program(caps, nchunk)
    in_maps, core_graphs = _host_inputs(
        V, graph_size, weight, bias, tune_weight, order, caps, offs, nchunk
    )
    if _run is None:
        _run = lambda nc, in_maps: run_bass_kernel_spmd(
            nc, in_maps, list(range(NCORES))
        ).results
    results = _run(nc, in_maps)
    return _assemble(results, core_graphs, caps, offs, nchunk, bias)



# revision 31
# speedup vs baseline: 2.2284x; 2.2284x over previous
"""Trainium2 Bass kernel for nn_MultiHeadGlobalAttention.

Math (B=64, N=4096, C=128, H=4):
  mask[b,n] = n < graph_size[b]
  Vg = (V @ weight + bias).reshape(B,N,H,C)
  a[b,n,h] = sum_c Vg[b,n,h,c] * tune[0,h,c]   -> leaky_relu -> masked softmax over n
  out[b] = (sum_n a[b,n,h] * Vg[b,n,h,:]).reshape(H*C)

Key reduction: softmax weights sum to 1, so
  out[b, h*C:(h+1)*C] = (sum_n e[n,h] * V[b,n,:]) / Z[b,h] @ W[:, h*C:(h+1)*C] + bias[h*C:(h+1)*C]
with logits l[n,h] = V[b,n,:] @ w2[:,h] + b2[h], w2 = sum_d W[:,h*C+d]*tune[h,d],
b2 = sum_d bias[h*C+d]*tune[h,d], e = exp(leaky_relu(l)) * mask01.
Vg is never materialized; V is read once per node.

Sharding: 64 graphs -> 8 cores x 8 slots. Graphs sorted by chunk count
(ceil(gs/128)) ascending; rank group g -> slot g, one graph per core.
Slot cap = max chunks in group -> uniform SPMD program across cores.

Device schedule (v2 — bf16 data plane, DMA-bound):
  - Everything on the data path is bf16 (V, mask, weights, e); PSUM
    accumulation stays fp32. Halves HBM traffic and runs PE at 1 c/row.
  - V shipped partition-major [128, nchunk*128]; few big DMAs (HWDGE
    setup is ~625ns each) with a small first and last group.
  - Per 8-chunk subgroup: PE transposes into a PSUM bank, one copyback
    to SBUF rotating DVE/ACT/DVE/Pool, then (two subgroups later, so the
    in-order PE queue never stalls on a copyback) per chunk one logit
    matmul (V^T stationary, w2 moving) + one K=1 bias matmul.
  - Per 32-chunk group: ACT Lrelu -> ACT Exp -> DVE mul by mask -> e.
    Accumulation matmuls (V stationary, e moving, 4 cycles each) one
    group late; per-slot Z via ones-matmul.
  - Single [32, 512] output matmul at the end; raw and Z partials are
    normalized (/Z, +bias) on the host.
"""

import math
import numpy as np

import concourse.bass as bass
import concourse.tile as tile
from concourse import bacc, mybir
from concourse.bass_utils import run_bass_kernel_spmd
from concourse.masks import make_identity

B, N, C, H = 64, 4096, 128, 4
P = 128          # chunk size == partition count
ALPHA = 0.2      # leaky_relu negative slope
NCORES = 8
NSLOTS = B // NCORES
SG = 4           # chunks per transpose subgroup (one PSUM bank of V^T)
GE = 32          # chunks per exp/softmax group
LLAG = 5         # subgroups between copyback and its logit matmuls
ELAG = 2         # subgroups between a group's last logits and its exp chain
ALAG = 8         # subgroups between a group's exp chain and its accum matmuls
F32 = mybir.dt.float32
BF16 = mybir.dt.bfloat16


def _plan(graph_size):
    """Sort graphs by chunk count ascending; rank group g -> slot g across
    cores. Ascending so the tail group is the big slot whose final chunks
    arrive last anyway."""
    nch = np.maximum(1, np.ceil(np.asarray(graph_size, np.int64) / P).astype(np.int64))
    order = np.argsort(nch, kind="stable")
    caps = [int(nch[order[g * NCORES + NCORES - 1]]) for g in range(NSLOTS)]
    offs = np.concatenate([[0], np.cumsum(caps)]).astype(np.int64)
    return order, caps, offs, int(offs[-1])


def _dma_sizes(nchunk):
    """V DMA group sizes: small head so transposes start early, 32-wide
    middle to amortize HWDGE setup, small final group for a short tail."""
    sizes = []
    rem = nchunk
    for s in (8, 8, 16):
        if rem <= 0:
            break
        t = min(s, rem)
        sizes.append(t)
        rem -= t
    while rem > 35:
        sizes.append(32)
        rem -= 32
    # shrinking tail: the last chunks arrive incrementally so the
    # transpose/copyback/logit chain overlaps the remaining transfers
    for s in (16, 8, 4, 4, 4, 4, 4, 4):
        if rem <= 3:
            break
        t = min(s, rem - 3)
        sizes.append(t)
        rem -= t
    if rem:
        sizes.append(rem)
    return sizes


def _build_program(caps, nchunk):
    nc = bacc.Bacc("TRN2", target_bir_lowering=False, debug=False)

    vh_d = nc.declare_dram_parameter("Vh", [P, nchunk * P], BF16, isOutput=False)
    mt_d = nc.declare_dram_parameter("mt", [2, nchunk * P], BF16, isOutput=False)
    wb_d = nc.declare_dram_parameter("wb", [C, 2 * H], BF16, isOutput=False)
    wm_d = nc.declare_dram_parameter("wmat", [C, H * C], BF16, isOutput=False)
    outb_d = nc.declare_dram_parameter("outb", [P, H * C + NSLOTS], F32, isOutput=True)

    offs = [0]
    for cp in caps:
        offs.append(offs[-1] + cp)
    slot_of = np.zeros(nchunk, np.int64)
    for g in range(NSLOTS):
        slot_of[offs[g]:offs[g + 1]] = g

    sizes = _dma_sizes(nchunk)
    gstart = [0]
    for s in sizes:
        gstart.append(gstart[-1] + s)
    ngd = len(sizes)

    nsub = (nchunk + SG - 1) // SG
    # exp groups: 32-wide in the body, shrinking at the tail so the final
    # exp->accum->output chain is short and pipelines with the last DMAs
    geb = [0]
    rem = nchunk
    while rem > 40:
        geb.append(geb[-1] + GE)
        rem -= GE
    while rem > 6:
        t = max(4, rem // 2)
        geb.append(geb[-1] + t)
        rem -= t
    if rem:
        geb.append(geb[-1] + rem)
    nge = len(geb) - 1
    ge_of = np.searchsorted(np.asarray(geb), np.arange(nchunk), side="right") - 1

    with tile.TileContext(nc) as tc:
        with (
            tc.tile_pool(name="consts", bufs=1) as consts,
            tc.tile_pool(name="vtsb", bufs=8) as vtsb_pool,
            tc.tile_pool(name="aexp", bufs=2) as a_pool,
            tc.tile_pool(name="outp", bufs=1) as out_pool,
            tc.tile_pool(name="ps_vt", bufs=5, space="PSUM") as ps_vt,
            tc.tile_pool(name="ps_l", bufs=2, space="PSUM") as ps_l,
            tc.tile_pool(name="ps_acc", bufs=1, space="PSUM") as ps_acc,
        ):
            # DMA order = criticality: first V chunks and wb gate the head of
            # the pipeline; em gates the first softmax; wm only the tail.
            vh_sb = consts.tile([P, nchunk * P], BF16)
            wb_sb = consts.tile([C, 2 * H], BF16)
            mt_sb = consts.tile([2, nchunk * P], BF16)
            wm_sb = consts.tile([C, H * C], BF16)

            def dma_vgroup(k):
                a = gstart[k] * P
                b = gstart[k + 1] * P
                nc.sync.dma_start(vh_sb[:, a:b], vh_d[:, a:b])

            # tiny param DMAs ride the Pool SWDGE queue: no HWDGE setup slot,
            # so the V stream on SP/HWDGE stays back-to-back
            dma_vgroup(0)
            nc.gpsimd.dma_start(wb_sb[:], wb_d[:])
            nc.gpsimd.dma_start(mt_sb[:], mt_d[:])
            nc.gpsimd.dma_start(wm_sb[:], wm_d[:])
            for k in range(1, ngd):
                dma_vgroup(k)

            w2_ap = wb_sb[:, 0:H]
            bm_ap = wb_sb[0:2, H:2 * H]   # row0 = b2, row1 = -100 (mask)

            ident = consts.tile([P, P], BF16)
            make_identity(nc, ident[:])
            ones = consts.tile([P, 1], BF16)
            nc.gpsimd.memset(ones[:], 1.0)
            zeros8 = consts.tile([P, NSLOTS], BF16)
            nc.gpsimd.memset(zeros8[:], 0.0)

            eall_sb = consts.tile([P, nchunk * H], BF16)
            acc_ps = ps_acc.tile([C, NSLOTS * H + NSLOTS], F32)
            psum_s = acc_ps[:, 0:NSLOTS * H]
            zc_ps = acc_ps[:, NSLOTS * H:]
            # zero-fill zc columns: rows above caps[s]*H are never written by
            # the Z matmuls but the merged tail copy reads the full tile
            nc.tensor.matmul(zc_ps[:], ident[:], zeros8[:], start=True, stop=True)
            sall = out_pool.tile([C, NSLOTS * H], BF16)
            outb_sb = out_pool.tile([P, H * C + NSLOTS], F32)
            nc.gpsimd.memset(outb_sb[:], 0.0)   # rows 32..128 of raw go out

            vt_tiles = {}    # subgroup k -> SBUF tile with V^T chunks
            lp_tiles = {}    # ge group -> PSUM logits tile
            cb_rot = [0]

            def vsl(j):
                return vh_sb[:, j * P:(j + 1) * P]

            def vt_slice(j):
                k = j // SG
                return vt_tiles[k][:, (j - k * SG) * P:(j - k * SG + 1) * P]

            def emit_T(k):
                c0, c1 = k * SG, min(nchunk, (k + 1) * SG)
                w = (c1 - c0) * P
                vt_ps = ps_vt.tile([P, SG * P], BF16)
                for j in range(c0, c1):
                    nc.tensor.transpose(
                        vt_ps[:, (j - c0) * P:(j - c0 + 1) * P], vsl(j), ident[:]
                    )
                vt_sb = vtsb_pool.tile([P, SG * P], BF16)
                # GPSIMD cannot read PSUM on TRN2: rotate DVE/ACT/DVE only
                r = cb_rot[0]
                cb_rot[0] = (r + 1) % 3
                if r == 1:
                    nc.scalar.activation(
                        vt_sb[:, :w], vt_ps[:, :w],
                        mybir.ActivationFunctionType.Copy,
                    )
                else:
                    nc.vector.tensor_copy(vt_sb[:, :w], vt_ps[:, :w])
                vt_tiles[k] = vt_sb

            def emit_L(k):
                c0, c1 = k * SG, min(nchunk, (k + 1) * SG)
                for j in range(c0, c1):
                    g = int(ge_of[j])
                    if j == geb[g]:
                        lp_tiles[g] = ps_l.tile([P, GE * H], F32, name="lps")
                    lp = lp_tiles[g]
                    off = (j - geb[g]) * H
                    nc.tensor.matmul(
                        lp[:, off:off + H], vt_slice(j), w2_ap,
                        start=True, stop=False,
                    )
                    # one K=2 matmul adds b2 (via the all-ones row of mt)
                    # and -100 to masked nodes (via the invalid row):
                    # exp(prelu(l - 100)) <= 3e-9 so padding drops out
                    nc.tensor.matmul(
                        lp[:, off:off + H], mt_sb[0:2, j * P:(j + 1) * P], bm_ap,
                        start=False, stop=True,
                    )

            def emit_exp(g):
                c0, c1 = geb[g], geb[g + 1]
                w = (c1 - c0) * H
                lp = lp_tiles[g]
                a_sb = a_pool.tile([P, GE * H], F32)
                # Prelu == leaky_relu; unlike Lrelu it shares the compiler's
                # exp_and_others act table, so ACT never reloads tables
                nc.scalar.activation(
                    a_sb[:, :w], lp[:, :w],
                    mybir.ActivationFunctionType.Prelu, alpha=ALPHA,
                )
                nc.scalar.activation(
                    eall_sb[:, c0 * H:c0 * H + w], a_sb[:, :w],
                    mybir.ActivationFunctionType.Exp,
                )

            def emit_A(g):
                # accumulation + Z matmuls for group g: PE ops gated only on
                # eall(g), which is ALAG subgroups old by the time these issue
                for j in range(geb[g], geb[g + 1]):
                    s = int(slot_of[j])
                    nc.tensor.matmul(
                        psum_s[:, s * H:(s + 1) * H], vsl(j),
                        eall_sb[:, j * H:(j + 1) * H],
                        start=(j == offs[s]), stop=(j == offs[s + 1] - 1),
                    )
                    if j == offs[s + 1] - 1:
                        # Z right after the slot closes: acc_ps has no open
                        # accumulation group at this point
                        lo = offs[s] * H
                        nc.tensor.matmul(
                            zc_ps[0:caps[s] * H, s:s + 1],
                            eall_sb[:, lo:lo + caps[s] * H],
                            ones[:], start=True, stop=True,
                        )

            exp_done = -1
            exp_ready = []   # (group, k at which its logits were emitted)
            chain_k = {}     # group -> k at which its exp chain was emitted
            a_next = [0]

            def flush_exp(now_k, force=False):
                # Emit exp chains only once their logits have had ELAG
                # subgroups to complete: ACT's in-order queue must never hold
                # a Prelu whose input isn't ready in front of a V^T copyback.
                nonlocal exp_done
                while exp_ready and (force or now_k - exp_ready[0][1] >= ELAG):
                    g, _ = exp_ready.pop(0)
                    emit_exp(g)
                    chain_k[g] = now_k
                    exp_done = g

            def flush_A(now_k, force=False):
                # A/Z only after the exp chain has had ALAG subgroups to
                # drain: PE's in-order queue must never stall on eall.
                while a_next[0] in chain_k and (
                    force or now_k - chain_k[a_next[0]] >= ALAG
                ):
                    emit_A(a_next[0])
                    a_next[0] += 1

            def after_L(upto_chunk, k):
                g = len(exp_ready) + exp_done
                while g + 1 < nge and geb[g + 2] <= upto_chunk:
                    g += 1
                    exp_ready.append((g, k))

            for k in range(nsub):
                flush_exp(k)
                flush_A(k)
                emit_T(k)
                if k >= LLAG:
                    emit_L(k - LLAG)
                    after_L(min(nchunk, (k - LLAG + 1) * SG), k)
            for k in range(max(0, nsub - LLAG), nsub):
                flush_exp(k + LLAG)
                flush_A(k + LLAG)
                emit_L(k)
                after_L(min(nchunk, (k + 1) * SG), k + LLAG)
            flush_exp(0, force=True)
            flush_A(0, force=True)

            # tail: gather S and Z, one output matmul, one merged DMA.
            # out_ps borrows the final vt slot (same 2KB bank; no transposes
            # follow, so no WAR chain behind it)
            out_ps = ps_vt.tile([NSLOTS * H, H * C], F32, name="vt_ps")
            nc.vector.tensor_copy(sall[:], psum_s[:])
            nc.scalar.activation(
                outb_sb[:, H * C:], zc_ps[:],
                mybir.ActivationFunctionType.Copy,
            )
            nc.tensor.matmul(
                out_ps[:], sall[:], wm_sb[:], start=True, stop=True
            )
            half = (H * C) // 2
            nc.vector.tensor_copy(
                outb_sb[0:NSLOTS * H, 0:half], out_ps[:, 0:half]
            )
            nc.scalar.activation(
                outb_sb[0:NSLOTS * H, half:H * C], out_ps[:, half:],
                mybir.ActivationFunctionType.Copy,
            )
            nc.sync.dma_start(outb_d[:], outb_sb[:])

    nc.compile()
    return nc


def _host_inputs(V, graph_size, weight, bias, tune_weight, order, caps, offs, nchunk):
    import ml_dtypes

    bf16 = ml_dtypes.bfloat16
    tw = np.asarray(tune_weight, np.float32)[0]                      # [H, C]
    wr = np.asarray(weight, np.float32).reshape(C, H, C)
    w2 = np.einsum("chd,hd->ch", wr, tw).astype(np.float32)          # [C, H]
    b2 = np.einsum("hd,hd->h", np.asarray(bias, np.float32).reshape(H, C), tw)

    wb = np.zeros((C, 2 * H), np.float32)
    wb[:, :H] = w2
    wb[0, H:2 * H] = b2        # pairs with the all-ones mt row
    wb[1, H:2 * H] = -100.0    # pairs with the invalid-flag mt row
    wb = wb.astype(bf16)
    wm = np.asarray(weight, np.float32).astype(bf16)

    gs = np.asarray(graph_size, np.int64)
    in_maps = []
    core_graphs = []
    for c in range(NCORES):
        graphs = [int(order[g * NCORES + c]) for g in range(NSLOTS)]
        core_graphs.append(graphs)
        vcat = np.concatenate(
            [V[b, : caps[g] * P, :] for g, b in enumerate(graphs)], axis=0
        ).astype(np.float32, copy=False)
        vh = np.ascontiguousarray(
            vcat.reshape(nchunk, P, C).transpose(1, 0, 2).reshape(P, nchunk * C)
        ).astype(bf16)
        # mt row 0 = ones (pairs with b2), row 1 = invalid flags (pairs
        # with -100); chunk j occupies columns j*128 .. +128
        mt = np.zeros((2, nchunk * P), np.float32)
        mt[0, :] = 1.0
        prow = np.arange(P)
        for g, b in enumerate(graphs):
            for j in range(caps[g]):
                jj = offs[g] + j
                mt[1, jj * P:(jj + 1) * P] = ((j * P + prow) >= gs[b])
        in_maps.append({"Vh": vh, "mt": mt.astype(bf16), "wb": wb, "wmat": wm})
    return in_maps, core_graphs


def _assemble(results, core_graphs, caps, offs, nchunk, bias):
    bias = np.asarray(bias, np.float32)
    out = np.empty((B, H * C), np.float32)
    for c in range(NCORES):
        outb = np.asarray(results[c]["outb"], np.float32)      # [P, H*C+NSLOTS]
        raw = outb[: NSLOTS * H, : H * C]
        zc = outb[:, H * C:]
        for g, b in enumerate(core_graphs[c]):
            Z = zc[: caps[g] * H, g].reshape(caps[g], H).sum(axis=0)   # [H]
            for h in range(H):
                blk = slice(h * C, (h + 1) * C)
                out[b, blk] = raw[g * H + h, blk] / Z[h] + bias[blk]
    return out


def kernel(V, graph_size, weight, bias, tune_weight, _run=None):
    order, caps, offs, nchunk = _plan(graph_size)
    nc = _build_program(caps, nchunk)
    in_maps, core_graphs = _host_inputs(
        V, graph_size, weight, bias, tune_weight, order, caps, offs, nchunk
    )
    if _run is None:
        _run = lambda nc, in_maps: run_bass_kernel_spmd(
            nc, in_maps, list(range(NCORES))
        ).results
    results = _run(nc, in_maps)
    return _assemble(results, core_graphs, caps, offs, nchunk, bias)
